# revision 9
# baseline (speedup 1.0000x reference)
"""Bass/Trainium2 kernel for nn_DiagonalTraining (per-anti-diagonal Linear).

Math: for each anti-diagonal i of x[B,S,S] (entries x[b,r,i-r], r<=i),
apply Linear_i (weights W[i,:i+1,:i+1], bias b[i,:i+1]) to the gathered
vector and scatter back reversed. Equivalent to:
    D[b,i,j] = x[b,j,i-j] (j<=i else 0)
    out[b,i,k] = sum_j W[i,k,j] * D[b,i,j] + b[i,k]
    new_x[b,r,c] = out[b,r+c,c] if r+c < S else x[b,r,c]

Device does the einsum (memory-bound: streams the valid triangle of W);
gather/scatter/bias are tiny O(S^2) host ops.

Sharding: interleaved over diagonals — core c owns i = c, c+8, ..., c+504
(slot m holds diagonal 8m+c, padded to length L=8(m+1)). All cores run one
identical SPMD program; padding rows/cols of W and D are zero by
construction so results are exact.

v4: W and D are fp8 e4m3 (W pre-scaled by 64 to stay in e4m3's normal
range; descaled on host). The gathered-D image and the W image live in
ONE dram tensor laid out in consumption order, fetched with 10 large
full-height consumption-contiguous DMAs alternating two HW queues — the
first transfer carries D plus the primer groups so the PE starts the
moment it lands, and the DMA byte rate (~0.29 ns/col) matches the PE's
4-way col-packed streaming rate (~0.25 ns/col) so the pipeline tracks
the stream end-to-end. Groups of 4 slots share a PSUM bank via
tile_position col-packing; PSUM->SBUF copies (vector, casting to bf16)
and per-pair output DMAs (gpsimd) drain behind the matmuls.
"""

import sys

sys.path.insert(0, "/opt/trn_rl_repo")

import numpy as np

B = 8
S = 512
NCORES = 8
M = 64  # diagonal slots per core
G = 16  # groups of 4 slots sharing a PSUM bank
LBAR = [8 * (m + 1) for m in range(M)]  # padded diagonal length per slot
NQG = [g // 4 + 1 for g in range(G)]
GCOLS = [NQG[g] * (128 * g + 80) for g in range(G)]  # w-image cols per group
LG = [32 * (g + 1) for g in range(G)]  # group output width
OCUM = np.cumsum([0] + LG).tolist()
OTOT = OCUM[G]  # 4352
DTCOLS = 4 * M * B  # 2048, the gathered-D image
WSCALE = 64.0
MODE = "fp8"  # informational; kernel always runs the fp8 scheme

# Consumption order: primer first, then largest-first so the PE tracks
# the arrival stream, tiny groups last (short tail).
ORDER = [3, 2, 15, 14, 13, 12, 11, 10, 9, 8, 7, 6, 5, 4, 1, 0]
# Image columns: D image first, then group images in consumption order.
WOFF = {}
_off = DTCOLS
for _g in ORDER:
    WOFF[_g] = _off
    _off += GCOLS[_g]
WTOT = _off  # 53888
# Fetch transfers: consumption-contiguous runs [start, end) in ORDER
# positions; the first also carries the D image. Alternating queues.
WDMA_SPLITS = [(0, 2), (2, 3), (3, 4), (4, 5), (5, 6), (6, 8), (8, 10),
               (10, 12), (12, 14), (14, 16)]
# Output DMA batches (consumption pairs, each contiguous in out dram).
OBATCH = [[3, 2], [15, 14], [13, 12], [11, 10], [9, 8], [7, 6], [5, 4], [1, 0]]

_compiled = {}


def build_program():
    """Build the SPMD Bass program (same instructions on all 8 cores)."""
    import concourse.mybir as mybir
    import concourse.tile as tile
    from concourse import bacc

    f8 = mybir.dt.float8e4
    f32 = mybir.dt.float32
    bf16 = mybir.dt.bfloat16

    nc = bacc.Bacc("TRN2")
    wimg = nc.dram_tensor("wimg", [128, WTOT], f8, kind="ExternalInput")
    out = nc.dram_tensor("out", [128, OTOT], bf16, kind="ExternalOutput")

    with tile.TileContext(nc) as tc:
        with (
            tc.tile_pool(name="wpool", bufs=1) as wpool,
            tc.tile_pool(name="opool", bufs=3) as opool,
            tc.tile_pool(name="psum", bufs=8, space="PSUM") as psum_pool,
        ):
            wtile = wpool.tile([128, WTOT], f8)
            w_engines = [nc.sync, nc.scalar]
            for i, (p0, p1) in enumerate(WDMA_SPLITS):
                a = 0 if p0 == 0 else WOFF[ORDER[p0]]
                b_ = WOFF[ORDER[p1 - 1]] + GCOLS[ORDER[p1 - 1]]
                eng = w_engines[i % 2]
                eng.dma_start(wtile[0:128, a:b_], wimg[0:128, a:b_])

            batch_of = {}
            for bi, batch in enumerate(OBATCH):
                for g in batch:
                    batch_of[g] = bi
            btiles = {}
            bdone = {bi: 0 for bi in range(len(OBATCH))}

            for g in ORDER:
                nq = NQG[g]
                ps = psum_pool.tile([128, 512], f32, tag="ps")
                for t in range(4):
                    m = 4 * g + t
                    L = LBAR[m]
                    woff = WOFF[g] + nq * sum(LBAR[4 * g : 4 * g + t])
                    for q in range(nq):
                        nc.tensor.matmul(
                            ps[32 * t : 32 * t + B, 0:L],
                            lhsT=wtile[
                                0:128, q * M * B + m * B : q * M * B + (m + 1) * B
                            ],
                            rhs=wtile[0:128, woff + q * L : woff + (q + 1) * L],
                            start=(q == 0),
                            stop=(q == nq - 1),
                            tile_position=(0, 32 * t),
                        )
                # Cast to bf16 into this group's slice of its batch tile.
                bi = batch_of[g]
                lo = min(OBATCH[bi])
                span = OCUM[max(OBATCH[bi]) + 1] - OCUM[lo]
                if bi not in btiles:
                    btiles[bi] = opool.tile(
                        [128, span], bf16, tag=f"ob{bi}", name=f"ob{bi}"
                    )
                bt = btiles[bi]
                off = OCUM[g] - OCUM[lo]
                nc.vector.tensor_copy(
                    bt[0:128, off : off + LG[g]], ps[0:128, 0 : LG[g]]
                )
                bdone[bi] += 1
                if bdone[bi] == len(OBATCH[bi]):
                    nc.gpsimd.dma_start(
                        out[:, OCUM[lo] : OCUM[lo] + span], bt[0:128, 0:span]
                    )

    nc.compile()
    return nc


def _get_program():
    if "fp8" not in _compiled:
        _compiled["fp8"] = build_program()
    return _compiled["fp8"]


def _prep_inputs(x, W):
    """Host-side shard prep: gather diagonals of x, pack W SBUF images."""
    import ml_dtypes

    f8 = np.dtype(ml_dtypes.float8_e4m3)
    i_idx = np.arange(S)[:, None]
    r_idx = np.arange(S)[None, :]
    cols = (i_idx - r_idx) % S
    valid = (r_idx <= i_idx)[None]
    D = np.where(valid, x[:, r_idx, cols], np.float32(0.0))  # [B, S(i), S(j)]
    Dq = D.astype(f8)
    Wq = (W * np.float32(WSCALE)).astype(f8)

    in_maps = []
    for c in range(NCORES):
        Wc = Wq[c::8]  # [M, S(k), S(j)]
        WIMG = np.empty((128, WTOT), dtype=f8)
        # D image: WIMG[j, q*M*B + m*B + b] = D[b, 8m+c, 128q+j]
        WIMG[:, 0:DTCOLS] = (
            Dq[:, c::8, :]
            .transpose(2, 1, 0)
            .reshape(4, 128, M, B)
            .transpose(1, 0, 2, 3)
            .reshape(128, DTCOLS)
        )
        for g in range(G):
            nq = NQG[g]
            col = WOFF[g]
            for t in range(4):
                m = 4 * g + t
                L = LBAR[m]
                # img[j, (q, k)] = Wc[m, k, 128q + j]
                blk = Wc[m, 0:L, 0 : 128 * nq]  # [k=L, j]
                img = (
                    blk.T.reshape(nq, 128, L).transpose(1, 0, 2).reshape(128, nq * L)
                )
                WIMG[:, col : col + nq * L] = img
                col += nq * L
        in_maps.append({"wimg": WIMG})
    return in_maps


def _postprocess(x, bvec, results):
    """Assemble per-core outputs, descale, add bias, scatter back."""
    out_full = np.empty((B, S, S), dtype=np.float32)
    for c in range(NCORES):
        o = np.asarray(results[c]["out"]).astype(np.float32)  # [128, OTOT]
        for g in range(G):
            blk = o[:, OCUM[g] : OCUM[g + 1]].reshape(4, 32, LG[g])[:, 0:B]
            for t in range(4):
                m = 4 * g + t
                out_full[:, 8 * m + c, 0 : LBAR[m]] = blk[t, :, 0 : LBAR[m]]
    out_full *= np.float32(1.0 / WSCALE)
    out_full += bvec[None]
    rr = np.arange(S)[:, None]
    cc = np.arange(S)[None, :]
    diag = rr + cc
    new_x = np.where(
        (diag < S)[None], out_full[:, np.minimum(diag, S - 1), cc], x
    ).astype(np.float32)
    return new_x


def kernel_run(x, W, b, mode=None, trace=False):
    from concourse.bass_utils import run_bass_kernel_spmd

    nc = _get_program()
    in_maps = _prep_inputs(x, W)
    res = run_bass_kernel_spmd(nc, in_maps, list(range(NCORES)), trace=trace)
    return _postprocess(x, b, res.results), res


def kernel(x, W, b):
    out, _ = kernel_run(np.asarray(x), np.asarray(W), np.asarray(b))
    return out
